# revision 39
# baseline (speedup 1.0000x reference)
"""Trainium2 Bass kernel for nn_MultiHeadAttention_83863531421896.

Full-input contract: kernel(**inputs) takes the unsharded tensors and
returns the full (2, 2048, 1024) output. Internally the 16 heads are
sharded 2-per-core across 8 NeuronCores (tensor parallel); each core
computes its heads' attention plus its slice of the output projection,
and the 8 partial projections are reduced on the host.

Per-core dataflow (heads h0, h1), bf16 matmul operands / f32 PSUM:
  qkv:   qT/kT/vT = W^T @ x^T per 512-token chunk (bf16, contraction
         over 8 k-tiles), bias added on DVE eviction; V^T transposed to
         [keys, hd] layout via one XBAR DMA transpose per (batch, head)
         into a contiguous staging tile, then DVE-copied into the
         [V | ones] stationary blocks.
  attn:  per (qchunk 512, head, kt-pair): S^T halves = K_kt Q^T into a
         2-bank PSUM tile, ONE exp over [128,1024] on ScalarE (softmax
         scale folded in), then AV with full 128-key contraction using
         stationary [V | ones] so softmax denominators accumulate in
         PSUM row 64 alongside the 64 output dims.
         Normalization: DVE evict [65,512], DRAM-bounce broadcast of
         the denominator row to 64 partitions, DVE recip + multiply.
  out:   out-proj partials [1024, tokens] f32, summed on host.
  The attention stream is ScalarE(exp)-bound, so next-batch qkv and
  prev-batch out-proj matmuls are interleaved into it one matmul per
  step to use the tensor engine's idle slots (engine queues are strict
  FIFO, so overlap must be explicit in program order).
"""

import sys

if "/opt/trn_rl_repo" not in sys.path:
    sys.path.insert(0, "/opt/trn_rl_repo")

import numpy as np

B = 2
S = 2048
D = 1024
H = 16
HD = 64
N_CORES = 8
HEADS_PER_CORE = H // N_CORES  # 2
M = B * S                      # 4096 tokens
N_MCHUNK_B = S // 512          # 4 m-chunks of 512 tokens per batch
N_KTILE = D // 128             # 8 contraction tiles for qkv
N_QCHUNK = S // 512            # 4 q-chunks per batch
N_KKTILE = S // 128            # 16 key tiles per batch
N_KTP = N_KKTILE // 2          # 8 key-tile pairs per batch
SCALE = 1.0 / np.sqrt(HD)

_CACHE = {}


def _build_module():
    import concourse.bass as bass
    import concourse.tile as tile
    from concourse import bacc, mybir

    f32 = mybir.dt.float32
    bf16 = mybir.dt.bfloat16
    Exp = mybir.ActivationFunctionType.Exp

    nc = bacc.Bacc("TRN2", target_bir_lowering=False, debug=False,
                   num_devices=N_CORES)

    xt_ap = nc.dram_tensor("xt", [D, M], bf16, kind="ExternalInput").ap()
    wqa_ap = nc.dram_tensor("wqa", [D, 128], bf16, kind="ExternalInput").ap()
    wqb_ap = nc.dram_tensor("wqb", [D, 128], bf16, kind="ExternalInput").ap()
    wv_ap = nc.dram_tensor("wv", [D, 128], bf16, kind="ExternalInput").ap()
    wo_ap = nc.dram_tensor("wo", [128, D], bf16, kind="ExternalInput").ap()
    ba_ap = nc.dram_tensor("ba", [128, 1], f32, kind="ExternalInput").ap()
    bb_ap = nc.dram_tensor("bb", [128, 1], f32, kind="ExternalInput").ap()
    bv_ap = nc.dram_tensor("bv", [128, 1], f32, kind="ExternalInput").ap()
    ones_ap = nc.dram_tensor("ones", [128, 64], bf16, kind="ExternalInput").ap()
    onesz_ap = nc.dram_tensor("onesz", [128, 64], bf16, kind="ExternalInput").ap()
    out_ap = nc.dram_tensor("partial", [D, M], f32, kind="ExternalOutput").ap()
    rscratch = nc.dram_tensor(
        "den_scratch", [B * N_QCHUNK * HEADS_PER_CORE, 512], f32).ap()

    with tile.TileContext(nc) as tc:
        with tc.tile_pool(name="persist", bufs=1) as persist, \
             tc.tile_pool(name="const", bufs=1) as const, \
             tc.tile_pool(name="xpool", bufs=4) as xpool, \
             tc.tile_pool(name="vt_pool", bufs=2) as vt_pool, \
             tc.tile_pool(name="ps8", bufs=1, space="PSUM") as ps8, \
             tc.tile_pool(name="epool", bufs=2) as epool, \
             tc.tile_pool(name="stage", bufs=2) as stage, \
             tc.tile_pool(name="fin", bufs=4) as fin:
            qka_sb = persist.tile([128, M], bf16, tag="qka")
            qkb_sb = persist.tile([128, M], bf16, tag="qkb")
            # V in [keys, hd] layout: [128 keys, b, kt, head, 64 hd | ones]
            v_sb = persist.tile([128, B, N_KKTILE, HEADS_PER_CORE, 65], bf16,
                                tag="vsb")
            outt_sb = persist.tile([128, M], bf16, tag="outt")

            # Startup order matters: the first V-projection group needs only
            # wv, bv, and xs[mc0]; everything else loads behind it (bulk of
            # batch-0 x on the scalar queue, idle until the first exp).
            wq_sb = [const.tile([128, N_KTILE, 128], bf16, tag=f"wq{i}",
                                name=f"wq{i}")
                     for i in range(3)]
            bv_sb = const.tile([128, 1], f32, tag="bv")
            nc.gpsimd.dma_start(bv_sb[:], bv_ap[:])
            nc.sync.dma_start(wq_sb[2][:],
                              wv_ap.rearrange("(k p) j -> p k j", k=N_KTILE))

            # one 3D-AP DMA per m-chunk (same (k p)->p k rearrange the
            # weight loads use): 4 issues per batch instead of 32
            xt_r = xt_ap.rearrange("(k p) m -> p k m", k=N_KTILE)

            def load_xs(b2, engs):
                xss = []
                for mc in range(N_MCHUNK_B):
                    mi = b2 * N_MCHUNK_B + mc
                    xs = xpool.tile([128, N_KTILE, 512], bf16, tag="xs",
                                    name=f"xs{mi}")
                    engs[mc % len(engs)].dma_start(
                        xs[:], xt_r[:, :, mi * 512:(mi + 1) * 512])
                    xss.append(xs)
                return xss

            xss0 = load_xs(0, [nc.sync, nc.gpsimd])

            ba_sb = const.tile([128, 1], f32, tag="ba")
            nc.gpsimd.dma_start(ba_sb[:], ba_ap[:])
            bb_sb = const.tile([128, 1], f32, tag="bb")
            nc.gpsimd.dma_start(bb_sb[:], bb_ap[:])
            nc.sync.dma_start(wq_sb[0][:],
                              wqa_ap.rearrange("(k p) j -> p k j", k=N_KTILE))
            nc.sync.dma_start(wq_sb[1][:],
                              wqb_ap.rearrange("(k p) j -> p k j", k=N_KTILE))
            # softmax-denominator ones column of the [V | ones] stationary
            nc.gpsimd.dma_start(
                v_sb[:, :, :, :, 64:65],
                ones_ap[:, 0:B * N_KKTILE * HEADS_PER_CORE].rearrange(
                    "p (b t h) -> p b t h", b=B, t=N_KKTILE)[:, :, :, :, None])
            wo_sb = const.tile([128, D], bf16, tag="wo")
            nc.gpsimd.dma_start(wo_sb[:], wo_ap[:])
            # PE denominator-broadcast stationary: row 64 = ones, all other
            # rows ZERO, so the 32-row PE tile contracts garbage moving rows
            # against zero weights (deterministic despite the padded tile)
            onesb_sb = const.tile([128, 64], bf16, tag="onesb")
            nc.gpsimd.dma_start(onesb_sb[:], onesz_ap[:])

            def qkv_ops(b2, xss, vt_sb, defer_q=False):
                """Emitter closures, each about one 512-row matmul of qkv
                work. V first so its transpose DMA can fire while the rest
                of the projection still interleaves; K before Q because the
                next batch's scores need all K but only Q's first m-chunk.
                """
                ops = []
                state = {}

                def transpose_v(b2=b2, vt_sb=vt_sb):
                    for h in range(HEADS_PER_CORE):
                        vstage = stage.tile([128, N_KKTILE, 64], bf16,
                                            tag="vstage", name=f"vs{b2}{h}")
                        nc.sync.dma_start_transpose(
                            vstage[:], vt_sb[h * 64:(h + 1) * 64, :])
                        nc.vector.tensor_copy(v_sb[:, b2, :, h, 0:64],
                                              vstage[:])

                for ei in (2, 1, 0):
                    bias, dest = ((ba_sb, qka_sb), (bb_sb, qkb_sb),
                                  (bv_sb, vt_sb))[ei]
                    for mc in range(N_MCHUNK_B):
                        for ki in range(N_KTILE):
                            def mm(ei=ei, mc=mc, ki=ki):
                                if ki == 0:
                                    state["ps"] = ps8.tile(
                                        [128, 512], f32, tag="bg", bufs=2,
                                        name=f"qkv{b2}{ei}{mc}")
                                nc.tensor.matmul(
                                    state["ps"][:], wq_sb[ei][:, ki],
                                    xss[mc][:, ki], start=(ki == 0),
                                    stop=(ki == N_KTILE - 1))
                            ops.append(mm)

                        def evict(ei=ei, mc=mc, bias=bias, dest=dest):
                            col = (b2 * N_MCHUNK_B + mc) if ei < 2 else mc
                            nc.vector.tensor_scalar_add(
                                dest[:, col * 512:(col + 1) * 512],
                                state["ps"][:], bias[:])
                            if ei == 2 and mc == N_MCHUNK_B - 1:
                                transpose_v()
                        ops.append(evict)
                if defer_q:
                    # V(36) K(36) Q-mc0(9) run now; Q mc1-3 interleave into
                    # attention (qchunk qi's scores only need Q m-chunk qi)
                    return ops[:81], ops[81:]
                return ops

            def outproj_ops(b2, mcs):
                ops = []
                state = {}
                for mc in mcs:
                    for et in range(D // 128):
                        def mm(b2=b2, mc=mc, et=et):
                            mrow = b2 * S + mc * 512
                            state["fp"] = ps8.tile([128, 512], f32, tag="bg",
                                                   bufs=2, name=f"fp{b2}{mc}{et}")
                            nc.tensor.matmul(state["fp"][:],
                                             wo_sb[:, et * 128:(et + 1) * 128],
                                             outt_sb[:, mrow:mrow + 512],
                                             start=True, stop=True)
                        def evict(b2=b2, mc=mc, et=et):
                            mrow = b2 * S + mc * 512
                            fo = fin.tile([128, 512], f32, tag="fo", name="fo")
                            if b2 == 1 and mc == N_MCHUNK_B - 1 and et % 2:
                                nc.scalar.activation(
                                    fo[:], state["fp"][:],
                                    mybir.ActivationFunctionType.Copy)
                            else:
                                nc.vector.tensor_copy(fo[:], state["fp"][:])
                            nc.sync.dma_start(
                                out_ap[et * 128:(et + 1) * 128, mrow:mrow + 512],
                                fo[:])
                        ops.append(mm)
                        ops.append(evict)
                return ops

            def attn_phase(b2, bg_ops, qchunk_done=None, bg_per_step=1):
                bg = list(bg_ops)
                pos = [0]

                def maybe_bg(n=1):
                    for _ in range(n):
                        if pos[0] >= len(bg):
                            return
                        bg[pos[0]]()
                        pos[0] += 1

                for qi in range(N_QCHUNK):
                    qcol = b2 * S + qi * 512
                    avp = [ps8.tile([128, 512], f32, tag=f"av{h}",
                                    name=f"av{b2}{qi}{h}")
                           for h in range(HEADS_PER_CORE)]

                    def emit_av(ktp, h, es):
                        first = (ktp == 0)
                        last = (ktp == N_KTP - 1)
                        for s in range(2):
                            nc.tensor.matmul(
                                avp[h][0:65, :],
                                v_sb[:, b2, 2 * ktp + s, h, :],
                                es[:, s, :],
                                start=(first and s == 0),
                                stop=(last and s == 1))

                    pending = None
                    for ktp in range(N_KTP):
                        for h in range(HEADS_PER_CORE):
                            sc = ps8.tile([128, 2, 512], f32, tag="sc",
                                          bufs=2, name=f"sc{ktp}{h}")
                            for s in range(2):
                                kkcol = b2 * S + (2 * ktp + s) * 128
                                nc.tensor.matmul(
                                    sc[:, s, :],
                                    qkb_sb[h * 64:(h + 1) * 64, kkcol:kkcol + 128],
                                    qka_sb[h * 64:(h + 1) * 64, qcol:qcol + 512],
                                    start=True, stop=True)
                            es = epool.tile([128, 2, 512], bf16, tag=f"e{h}",
                                            bufs=3, name=f"e{ktp}{h}")
                            nc.scalar.activation(es[:], sc[:], Exp, scale=SCALE)
                            if pending is not None:
                                emit_av(*pending)
                            maybe_bg(bg_per_step)
                            pending = (ktp, h, es)
                    emit_av(*pending)

                    pe_bcast = False  # PE-broadcast fast path NaNs on hw
                    for h in range(HEADS_PER_CORE):
                        # evict promptly so avp banks free for next qchunk
                        st = stage.tile([128, 512], f32, tag="st", name="st")
                        nc.vector.tensor_copy(st[0:65, :], avp[h][0:65, :])
                        if pe_bcast:
                            # low-latency path for the final normalizes: the
                            # DRAM-bounce broadcast would sit on the critical
                            # path, so broadcast 1/denom on the PE instead,
                            # reusing the just-freed av bank.
                            rc = stage.tile([128, 512], f32, tag="rc",
                                            name="rc")
                            nc.vector.reciprocal_approx_fast(rc[64:65, :],
                                                            st[64:65, :])
                            rcb = stage.tile([128, 512], bf16, tag="rcb",
                                             name="rcb")
                            # rows 65:96 feed the padded 32-row PE tile;
                            # garbage there can be NaN and 0*NaN = NaN
                            nc.gpsimd.memset(rcb[64:96, :], 0.0)
                            nc.vector.tensor_copy(rcb[64:65, :], rc[64:65, :])
                            rbp = ps8.tile([128, 512], f32, tag=f"av{h}",
                                           name=f"rbp{h}")
                            nc.tensor.matmul(rbp[0:64, :], onesb_sb[64:96, :],
                                             rcb[64:96, :], start=True,
                                             stop=True)
                            rb2 = rbp
                        else:
                            sidx = (b2 * N_QCHUNK + qi) * HEADS_PER_CORE + h
                            beng = (nc.sync if (b2 == 1 and h == 1
                                                and qi >= N_QCHUNK - 2)
                                    else nc.gpsimd)
                            beng.dma_start(rscratch[sidx:sidx + 1, :],
                                           st[64:65, :])
                            rb = stage.tile([128, 512], f32, tag="rb",
                                            name="rb")
                            beng.dma_start(
                                rb[0:64, :],
                                rscratch[sidx:sidx + 1, :]
                                .partition_broadcast(64).squeeze(1))
                            rb2 = stage.tile([128, 512], f32, tag="rb2",
                                             name="rb2")
                            nc.vector.reciprocal_approx_fast(rb2[0:64, :],
                                                            rb[0:64, :])
                        if h == 0:
                            nc.vector.tensor_mul(outt_sb[0:64, qcol:qcol + 512],
                                                 st[0:64, :], rb2[0:64, :])
                        else:
                            tm = stage.tile([64, 512], bf16, tag="tm",
                                            name="tm")
                            nc.vector.tensor_mul(tm[0:64, :], st[0:64, :],
                                                 rb2[0:64, :])
                            nc.gpsimd.dma_start(
                                outt_sb[64:128, qcol:qcol + 512], tm[0:64, :])
                    if qchunk_done is not None:
                        bg.extend(qchunk_done(qi))
                    maybe_bg(4)

                while pos[0] < len(bg):
                    bg[pos[0]]()
                    pos[0] += 1

            # batch 0 projection (nothing to hide it behind)
            vt0 = vt_pool.tile([128, S], bf16, tag="vt", name="vt0")
            for op in qkv_ops(0, xss0, vt0):
                op()
            # batch 1 qkv interleaves into batch 0 attention
            xss1 = load_xs(1, [nc.sync, nc.gpsimd])
            vt1 = vt_pool.tile([128, S], bf16, tag="vt", name="vt1")
            attn_phase(0, qkv_ops(1, xss1, vt1), bg_per_step=1)
            tc.no_sync_barrier()
            # batch 0 out-proj (and batch 1's, per finished qchunk)
            # interleaves into batch 1 attention
            attn_phase(1, outproj_ops(0, range(N_MCHUNK_B)),
                       qchunk_done=lambda qi: outproj_ops(1, [qi]),
                       bg_per_step=2)
            tc.no_sync_barrier()
    nc.compile()
    return nc


def _shard_inputs(x, w_qkv, b_qkv, w_out):
    import ml_dtypes
    bf16 = ml_dtypes.bfloat16

    xt = np.ascontiguousarray(x.reshape(M, D).T.astype(bf16))  # (1024, 4096)
    ones = np.ones((128, 64), dtype=bf16)
    onesz = np.zeros((128, 64), dtype=bf16)
    onesz[64, :] = 1.0
    in_maps = []
    for c in range(N_CORES):
        h0 = HEADS_PER_CORE * c
        rows_q, rows_k, rows_v, dcols = [], [], [], []
        for h in (h0, h0 + 1):
            rows_q += list(range(h * 192, h * 192 + 64))
            rows_k += list(range(h * 192 + 64, h * 192 + 128))
            rows_v += list(range(h * 192 + 128, h * 192 + 192))
            dcols += list(range(h * 64, (h + 1) * 64))
        in_maps.append({
            "xt": xt,
            "wqa": np.ascontiguousarray(w_qkv[rows_q, :].T.astype(bf16)),
            "wqb": np.ascontiguousarray(w_qkv[rows_k, :].T.astype(bf16)),
            "wv": np.ascontiguousarray(w_qkv[rows_v, :].T.astype(bf16)),
            "wo": np.ascontiguousarray(w_out[:, dcols].T.astype(bf16)),
            "ba": np.ascontiguousarray(b_qkv[rows_q].reshape(128, 1)),
            "bb": np.ascontiguousarray(b_qkv[rows_k].reshape(128, 1)),
            "bv": np.ascontiguousarray(b_qkv[rows_v].reshape(128, 1)),
            "ones": ones,
            "onesz": onesz,
        })
    return in_maps


def kernel(x, w_qkv, b_qkv, w_out, b_out, _trace=False):
    from concourse.bass_utils import run_bass_kernel_spmd

    x = np.asarray(x, dtype=np.float32)
    w_qkv = np.asarray(w_qkv, dtype=np.float32)
    b_qkv = np.asarray(b_qkv, dtype=np.float32)
    w_out = np.asarray(w_out, dtype=np.float32)
    b_out = np.asarray(b_out, dtype=np.float32)

    if "nc" not in _CACHE:
        _CACHE["nc"] = _build_module()
    nc = _CACHE["nc"]

    in_maps = _shard_inputs(x, w_qkv, b_qkv, w_out)
    res = run_bass_kernel_spmd(nc, in_maps, list(range(N_CORES)), trace=_trace)
    acc = np.zeros((D, M), dtype=np.float64)
    for c in range(N_CORES):
        acc += res.results[c]["partial"]
    acc = acc.T + b_out
    out = acc.astype(np.float32).reshape(B, S, D)
    if _trace:
        _CACHE["last_exec_time_ns"] = res.exec_time_ns
        _CACHE["last_res"] = res
    return out
